# revision 4
# baseline (speedup 1.0000x reference)
"""GRU (nn_ControllerLatent) Trainium2 kernel.

Reference math: single GRU over S = T*L = 1024 timesteps, batch N=64,
D = H = 1024.  Data-parallel over batch across 8 NeuronCores (8 batch
elems/core).  Per core, everything is kept gate-major ("transposed"):
  h is stored as hT tiles (128 H-partition, 8 batch cols),
  gates are computed as ghT = W_hh @ hT via 24 m-tiles x 8 k-tiles
  of (128,128) stationary weights against (128,8) moving h tiles.
Phase A precomputes the input projections gi = W_ih @ x (+biases) for
all steps into HBM (bf16); Phase B runs the sequential recurrence.
"""

import os
import numpy as np
import ml_dtypes

import concourse.bass as bass
import concourse.bacc as bacc
import concourse.tile as tile
import concourse.mybir as mybir
from concourse import bass_utils

L, N, T, D, H = 8, 64, 128, 1024, 1024
S = T * L            # 1024 sequential steps
NCORES = 8
NL = N // NCORES     # 8 batch elements per core
G = 3 * H            # 3072 gate rows
MT = G // 128        # 24 gate m-tiles
KT = H // 128        # 8 contraction k-tiles
CH = 64              # steps per phase-A chunk
NCH = S // CH        # 16 chunks
U = 4                # steps per phase-B inner loop body

F32 = mybir.dt.float32
BF16 = mybir.dt.bfloat16
AF = mybir.ActivationFunctionType

_cache = {}


def _build(repeat=1, s_steps=S, num_devices=NCORES):
    nch = s_steps // CH
    nc = bacc.Bacc("TRN2", target_bir_lowering=False, debug=False,
                   num_devices=num_devices)

    xsT = nc.dram_tensor("xsT", [KT, 128, s_steps * NL], BF16, kind="ExternalInput")
    wih = nc.dram_tensor("wihT", [KT, 128, G], BF16, kind="ExternalInput")
    whh = nc.dram_tensor("whhT", [KT, 128, G], BF16, kind="ExternalInput")
    # combined bias (b_ih + b_hh for r,z rows; b_ih for n rows), per gate row
    bias_c = nc.dram_tensor("bias_c", [MT, 128], F32, kind="ExternalInput")
    # b_hh n-gate part, broadcast over batch on host: [k][p][b]
    bias_hn = nc.dram_tensor("bias_hn", [128, KT * NL], F32, kind="ExternalInput")
    h0 = nc.dram_tensor("h0", [128, KT * NL], F32, kind="ExternalInput")
    gi_hbm = nc.dram_tensor("gi_scratch", [nch, MT * 128 * CH * NL], BF16)
    outs = nc.dram_tensor("outs", [s_steps, KT * 128 * NL], F32, kind="ExternalOutput")

    with tile.TileContext(nc) as tc:
        with (
            tc.tile_pool(name="wpool", bufs=1) as wpool,
            tc.tile_pool(name="spool", bufs=1) as spool,
            tc.tile_pool(name="xpool", bufs=2) as xpool,
            tc.tile_pool(name="gia", bufs=3) as giapool,
            tc.tile_pool(name="psa", bufs=2, space="PSUM") as psapool,
            tc.tile_pool(name="gpool", bufs=1) as gpool,
            tc.tile_pool(name="ps", bufs=2, space="PSUM") as pspool,
            tc.tile_pool(name="et", bufs=2) as etpool,
        ):
            whh_sb = wpool.tile([128, KT * G], BF16, tag="whh")
            wih_sb = wpool.tile([128, KT * G], BF16, tag="wih")
            bias_sb = spool.tile([128, MT], F32, tag="biasc")
            bhn_sb = spool.tile([128, KT * NL], F32, tag="bhn")
            h_t = [spool.tile([128, KT * NL], F32, tag=f"h{p}", name=f"h{p}")
                   for p in (0, 1)]
            hbf = [spool.tile([128, KT * NL], BF16, tag=f"hb{p}", name=f"hb{p}")
                   for p in (0, 1)]

            for k in range(KT):
                nc.sync.dma_start(whh_sb[:, k * G:(k + 1) * G], whh[k, :, :])
                nc.sync.dma_start(wih_sb[:, k * G:(k + 1) * G], wih[k, :, :])
            nc.sync.dma_start(
                bias_sb[:, :], bias_c.ap().rearrange("m p -> p m"))
            nc.sync.dma_start(bhn_sb[:, :], bias_hn.ap())
            nc.sync.dma_start(h_t[0][:, :], h0.ap())
            nc.vector.tensor_copy(hbf[0][:, :], h_t[0][:, :])

            # ---------------- Phase A: gi = W_ih @ x + bias ----------------
            with tc.For_i(0, nch) as c:
                xs_sb = xpool.tile([128, KT * CH * NL], BF16, tag="xs")
                for k in range(KT):
                    nc.sync.dma_start(
                        xs_sb[:, k * 512:(k + 1) * 512],
                        xsT[k, :, bass.ds(c * 512, 512)])
                for m in range(MT):
                    psa = psapool.tile([128, 512], F32, tag="psa")
                    for k in range(KT):
                        nc.tensor.matmul(
                            psa[:, :],
                            wih_sb[:, k * G + m * 128: k * G + (m + 1) * 128],
                            xs_sb[:, k * 512:(k + 1) * 512],
                            start=(k == 0), stop=(k == KT - 1))
                    gia = giapool.tile([128, 512], BF16, tag="gia")
                    nc.scalar.activation(gia[:, :], psa[:, :], AF.Identity,
                                         bias=bias_sb[:, m:m + 1])
                    dst = gi_hbm.ap()[bass.ds(c, 1),
                                      m * 65536:(m + 1) * 65536]
                    nc.sync.dma_start(
                        dst.rearrange("a (p s) -> (a p) s", p=128), gia[:, :])

            # ---------------- Phase B: recurrence -------------------------
            # gate m-tile order within psum groups: r = 0..7, z = 8..15,
            # n = 16..23.  h-slice (k-tile) j corresponds to n m-tile 16+j.
            with tc.For_i(0, nch, name="chunk") as c:
                gi_sb = gpool.tile([128, MT * CH * NL], BF16, tag="gi")
                src = gi_hbm.ap()[bass.ds(c, 1), :]
                nc.sync.dma_start(
                    gi_sb[:, :],
                    src.rearrange("a (m p s) -> (a p) m s", m=MT, p=128))
                gi_v = gi_sb[:, :].rearrange("p (m s b) -> p m s b", m=MT, b=NL)
                with tc.For_i(0, CH // U, name="steps") as j:
                    for u in range(U):
                        sidx = j * U + u
                        pr = u % 2        # parity holding h_{t-1}
                        pw = (u + 1) % 2  # parity receiving h_t
                        ps_r = pspool.tile([128, NL * 8], F32, tag="psr")
                        ps_z = pspool.tile([128, NL * 8], F32, tag="psz")
                        ps_n = pspool.tile([128, NL * 8], F32, tag="psn")
                        for grp, ps in ((0, ps_r), (2, ps_n), (1, ps_z)):
                            for m8 in range(8):
                                m = grp * 8 + m8
                                for k in range(KT):
                                    nc.tensor.matmul(
                                        ps[:, m8 * NL:(m8 + 1) * NL],
                                        whh_sb[:, k * G + m * 128:
                                               k * G + (m + 1) * 128],
                                        hbf[pr][:, k * NL:(k + 1) * NL],
                                        start=(k == 0), stop=(k == KT - 1))
                        # eltwise
                        rpre = etpool.tile([128, 64], F32, tag="rpre")
                        zpre = etpool.tile([128, 64], F32, tag="zpre")
                        r_sb = etpool.tile([128, 64], F32, tag="r")
                        z_sb = etpool.tile([128, 64], F32, tag="z")
                        t1 = etpool.tile([128, 64], F32, tag="t1")
                        t2 = etpool.tile([128, 64], F32, tag="t2")
                        npre = etpool.tile([128, 64], F32, tag="npre")
                        n_sb = etpool.tile([128, 64], F32, tag="n")
                        d_sb = etpool.tile([128, 64], F32, tag="d")
                        e_sb = etpool.tile([128, 64], F32, tag="e")
                        nc.vector.tensor_add(
                            rpre[:, :], ps_r[:, :],
                            gi_v[:, 0:8, bass.ds(sidx, 1), :])
                        nc.scalar.activation(r_sb[:, :], rpre[:, :], AF.Sigmoid)
                        nc.vector.tensor_add(t1[:, :], ps_n[:, :], bhn_sb[:, :])
                        nc.vector.tensor_mul(t2[:, :], r_sb[:, :], t1[:, :])
                        nc.vector.tensor_add(
                            npre[:, :], t2[:, :],
                            gi_v[:, 16:24, bass.ds(sidx, 1), :])
                        nc.scalar.activation(n_sb[:, :], npre[:, :], AF.Tanh)
                        nc.vector.tensor_add(
                            zpre[:, :], ps_z[:, :],
                            gi_v[:, 8:16, bass.ds(sidx, 1), :])
                        nc.scalar.activation(z_sb[:, :], zpre[:, :], AF.Sigmoid)
                        nc.vector.tensor_sub(d_sb[:, :], h_t[pr][:, :], n_sb[:, :])
                        nc.vector.tensor_mul(e_sb[:, :], z_sb[:, :], d_sb[:, :])
                        nc.vector.tensor_add(h_t[pw][:, :], n_sb[:, :], e_sb[:, :])
                        nc.vector.tensor_copy(hbf[pw][:, :], h_t[pw][:, :])
                        dst = outs.ap()[bass.ds(c * CH + j * U + u, 1), :]
                        nc.sync.dma_start(
                            dst.rearrange("a (k p b) -> (a p) k b", k=KT, p=128),
                            h_t[pw][:, :])

    nc.compile()
    return nc


def _prep_inputs(x, rnn_state, W_ih, W_hh, b_ih, b_hh):
    bf = ml_dtypes.bfloat16
    xs = np.ascontiguousarray(np.transpose(x, (2, 0, 1, 3))).reshape(S, N, D)
    whhT = np.ascontiguousarray(W_hh.T).reshape(KT, 128, G).astype(bf)
    wihT = np.ascontiguousarray(W_ih.T).reshape(KT, 128, G).astype(bf)
    bias_c = (b_ih + np.concatenate([b_hh[:2 * H], np.zeros(H, np.float32)])
              ).astype(np.float32).reshape(MT, 128)
    bias_hn = np.repeat(b_hh[2 * H:].reshape(KT, 128, 1), NL, axis=2)
    bias_hn = np.ascontiguousarray(bias_hn.transpose(1, 0, 2)).reshape(128, KT * NL)
    in_maps = []
    for c in range(NCORES):
        xs_c = xs[:, c * NL:(c + 1) * NL, :]             # (S, 8, D)
        xsT_c = np.ascontiguousarray(xs_c.transpose(2, 0, 1)).reshape(
            KT, 128, S * NL).astype(bf)
        st = rnn_state[0, c * NL:(c + 1) * NL, :]        # (8, H)
        h0c = np.ascontiguousarray(
            st.T.reshape(KT, 128, NL).transpose(1, 0, 2)).reshape(128, KT * NL)
        in_maps.append({
            "xsT": xsT_c, "wihT": wihT, "whhT": whhT,
            "bias_c": bias_c, "bias_hn": bias_hn.astype(np.float32),
            "h0": h0c.astype(np.float32),
        })
    return in_maps


def kernel(x, rnn_state, W_ih, W_hh, b_ih, b_hh):
    x = np.asarray(x, np.float32)
    rnn_state = np.asarray(rnn_state, np.float32)
    W_ih = np.asarray(W_ih, np.float32)
    W_hh = np.asarray(W_hh, np.float32)
    b_ih = np.asarray(b_ih, np.float32)
    b_hh = np.asarray(b_hh, np.float32)

    if "nc" not in _cache:
        _cache["nc"] = _build()
    nc = _cache["nc"]
    in_maps = _prep_inputs(x, rnn_state, W_ih, W_hh, b_ih, b_hh)
    res = bass_utils.run_bass_kernel_spmd(nc, in_maps,
                                          core_ids=list(range(NCORES)))
    output = np.empty((N, S, H), np.float32)
    for c in range(NCORES):
        o = res.results[c]["outs"].reshape(S, KT, 128, NL)
        # out[n, t, h]: n = c*NL + b, h = k*128 + p
        output[c * NL:(c + 1) * NL] = o.transpose(3, 0, 1, 2).reshape(NL, S, H)
    h_last = output[:, -1, :].copy()
    return output, h_last[None]


if __name__ == "__main__":
    import jax
    import reference

    inputs = {k: np.asarray(v) for k, v in reference.setup_inputs().items()}
    out, hl = kernel(**inputs)
    exp_out, exp_hl = reference.reference(**reference.setup_inputs())
    exp_out = np.asarray(exp_out)
    rel = np.linalg.norm(out - exp_out) / np.linalg.norm(exp_out)
    print("Relative error:", rel)
    print("h_last rel:", np.linalg.norm(hl - np.asarray(exp_hl)) /
          max(np.linalg.norm(np.asarray(exp_hl)), 1e-9))
